# revision 15
# baseline (speedup 1.0000x reference)
"""Trainium2 Bass kernel for the patch-GP conditional (conv GP layer).

Contract: kernel(**inputs) takes the FULL inputs (as produced by
setup_inputs()) and returns the FULL output (mean, var), each [N, P*G].

Math (equivalent to the reference's whitened-free conditional):
    Kuf[g,m,x]  = variance * exp(-0.5*(||z_m||^2 + ||x_x||^2 - 2 z_m.x_x)/ls^2)
                = cs[x] * kt[g,m,x],   cs[x] = exp(-0.5*||x_x||^2/ls^2)
    kt[g,m,x]   = exp(-0.5*(||z_m||^2 - 2 z_m.x_x)/ls^2 + ln(variance))
    fmean[g,x]  = cs[x] * sum_m d_g[m] kt[g,m,x],   d_g = Kuu_g^{-1} q_mu[:,g]
    fvar[g,x]   = variance - cs[x]^2 * sum_k kt[g,k,x] (Q_g @ kt[g])[k,x]
    Q_g         = Kuu_g^{-1} - (Kuu_g^{-1} Lq_g)(Kuu_g^{-1} Lq_g)^T
Host does the tiny O(M^3) prep in float64 (Kuu, inverse, Q, d), the patch
extraction / layout, and the final per-column cs/cs^2 rescale; the 8
NeuronCores each do the O(M * Ploc*N) work for their shard of P.

Precision split (hybrid mode): the mean path cancels heavily, so it runs
in float32r (sq matmuls, kt storage, d^T kt); the variance quad form is
benign, so Q @ kt and the ones-reduce run in bf16 (kt copied to bf16 on
the otherwise-idle GpSimd engine).

Device per core (x = ploc*N + n, Xloc = 98*32 = 3136 columns):
    sq    = zsb[:,g,mt]^T @ xaug          (3 f32r matmuls / (g,chunk), K=75)
    kt_r  = exp(scale*sq + bias_m)        (ACT, per-partition bias, f32r)
    kt_b  = bf16(kt_r)                    (GpSimd copy)
    R     = Q @ kt_b                      (9 bf16 matmuls / (g,chunk))
    pacc  = sum_kt (kt_b .* R)            (DVE mul+add, bf16)
    pv    = ones^T pacc                   (1 bf16 matmul)
    pm    = d^T kt_r                      (3 f32r matmuls)
    out rows: [pm_g0, pm_g1, pv_g0, pv_g1]  (raw, host rescales)
"""

import numpy as np

# Problem constants (hardcoded per the task contract).
H = 32
W = 32
C = 3
PH = 5
PW = 5
JITTER = 1e-6
N = 32
G = 2
M = 384
L = PH * PW * C  # 75
P = (H - PH + 1) * (W - PW + 1)  # 784
NCORES = 8
PLOC = P // NCORES  # 98
XL = PLOC * N  # 3136
CHW = 512  # max free-dim chunk width (PSUM bank holds 512 fp32)
CHUNKS = [(i * CHW, CHW) for i in range(XL // CHW)] + (
    [(XL - XL % CHW, XL % CHW)] if XL % CHW else []
)
NCH = len(CHUNKS)  # 7 (6x512 + 1x64)
MT = M // 128  # 3 partition tiles of the inducing dim
WARM_MM = 20  # PE warmup matmuls issued during the input DMA phase

# "hybrid" (default): mean path f32r, var quad form bf16.
# "f32r": everything f32r. "bf16": everything bf16 (fast, less accurate).
MODE = "fp16"

_CACHE = {}


def _ensure_concourse():
    try:
        import concourse  # noqa: F401
    except ImportError:
        import sys

        for p in ("/opt/trn_rl_repo", "/root/.axon_site/_ro/trn_rl_repo"):
            if p not in sys.path:
                sys.path.insert(0, p)


def _np_dts(mode):
    import ml_dtypes

    bf, f32 = ml_dtypes.bfloat16, np.float32
    if mode == "bf16":
        return bf, bf
    if mode == "f32r":
        return f32, f32
    if mode == "fp16":
        return np.float16, np.float16
    return f32, bf  # hybrid: (accurate, fast)


def _build(scale_imm: float, mode: str):
    """Build + compile the single-core SPMD program (same NEFF on all cores)."""
    _ensure_concourse()
    from concourse import bacc, mybir, tile

    f32 = mybir.dt.float32
    bf16 = mybir.dt.bfloat16
    f32r = mybir.dt.float32r
    if mode == "bf16":
        DTA = DTB = bf16
    elif mode == "f32r":
        DTA = DTB = f32r
    elif mode == "fp16":
        DTA = DTB = mybir.dt.float16
    else:
        DTA, DTB = f32r, bf16
    split = DTA != DTB
    EXP = mybir.ActivationFunctionType.Exp

    nc = bacc.Bacc("TRN2", target_bir_lowering=False, debug=False)

    xt = nc.dram_tensor("xt", [L, XL], DTA, kind="ExternalInput").ap()
    zaug = nc.dram_tensor("zaug", [L, G, M], DTA, kind="ExternalInput").ap()
    qmat = nc.dram_tensor("qmat", [128, G, MT, M], DTB, kind="ExternalInput").ap()
    dv = nc.dram_tensor("dv", [128, G * MT], DTA, kind="ExternalInput").ap()
    bv = nc.dram_tensor("bv", [128, G * MT], f32, kind="ExternalInput").ap()
    ones = nc.dram_tensor("ones", [128, 1], DTB, kind="ExternalInput").ap()
    out = nc.dram_tensor("out", [2 * G, XL], f32, kind="ExternalOutput").ap()

    with tile.TileContext(nc) as tc:
        with (
            tc.tile_pool(name="const", bufs=1) as const,
            tc.tile_pool(name="work", bufs=2) as work,
            tc.tile_pool(name="ps", bufs=2, space="PSUM") as ps,
        ):
            # PE warmup: dense dummy matmuls with no input deps, issued
            # while the input DMAs are in flight, so the HAM clock gate
            # reaches 8/8 before the real matmuls start.
            wsrc = const.tile([128, CHW], bf16)
            nc.vector.memset(wsrc, 0.0)
            for _ in range(WARM_MM):
                wps = ps.tile([128, CHW], f32, tag="psq", name="wps", bufs=3)
                nc.tensor.matmul(wps, wsrc[:, 0:128], wsrc)

            zsb = const.tile([L, G, M], DTA)
            nc.sync.dma_start(out=zsb, in_=zaug)
            bsb = const.tile([128, G * MT], f32)
            nc.sync.dma_start(out=bsb, in_=bv)
            xaug = const.tile([L, XL], DTA)
            nc.sync.dma_start(out=xaug[:, 0:CHW], in_=xt[:, 0:CHW])
            qsb = const.tile([128, G, MT, M], DTB)
            nc.sync.dma_start(out=qsb[:, 0], in_=qmat[:, 0])
            dsb = const.tile([128, G * MT], DTA)
            nc.sync.dma_start(out=dsb, in_=dv)
            osb = const.tile([128, 1], DTB)
            nc.sync.dma_start(out=osb, in_=ones)
            for off, cw in CHUNKS[1:]:
                csl = slice(off, off + cw)
                nc.sync.dma_start(out=xaug[:, csl], in_=xt[:, csl])
            nc.sync.dma_start(out=qsb[:, 1], in_=qmat[:, 1])

            macc = [const.tile([1, XL], f32, name=f"macc{g}") for g in range(G)]
            vacc = [const.tile([1, XL], f32, name=f"vacc{g}") for g in range(G)]

            for g in range(G):
                for off, cw in CHUNKS:
                    sl = slice(off, off + cw)
                    kufr = []
                    kufb = []
                    for mt in range(MT):
                        psq = ps.tile([128, CHW], f32, tag="psq", name="psq", bufs=3)[
                            :, :cw
                        ]
                        nc.tensor.matmul(
                            psq,
                            zsb[:, g, mt * 128 : (mt + 1) * 128],
                            xaug[:, sl],
                        )
                        kr = work.tile([128, CHW], DTA, tag=f"kr{mt}", name=f"kr{mt}")[
                            :, :cw
                        ]
                        nc.scalar.activation(
                            kr,
                            psq,
                            EXP,
                            bias=bsb[:, g * MT + mt : g * MT + mt + 1],
                            scale=scale_imm,
                        )
                        kufr.append(kr)
                        if split:
                            kb = work.tile(
                                [128, CHW], DTB, tag=f"kb{mt}", name=f"kb{mt}"
                            )[:, :cw]
                            nc.gpsimd.tensor_copy(out=kb, in_=kr.bitcast(f32))
                            kufb.append(kb)
                        else:
                            kufb.append(kr)
                    pacc = work.tile([128, CHW], DTB, tag="pacc", name="pacc")[
                        :, :cw
                    ]
                    pmp = ps.tile([1, CHW], f32, tag="pmp", name="pmp")[:, :cw]
                    for kt in range(MT):
                        pr = ps.tile([128, CHW], f32, tag="pr", name="pr")[:, :cw]
                        for mt in range(MT):
                            nc.tensor.matmul(
                                pr,
                                qsb[:, g, mt, kt * 128 : (kt + 1) * 128],
                                kufb[mt],
                                start=(mt == 0),
                                stop=(mt == MT - 1),
                            )
                        # cheap-LDW pm matmul between R groups hides the
                        # next group's weight load behind its stream
                        nc.tensor.matmul(
                            pmp,
                            dsb[:, g * MT + kt : g * MT + kt + 1],
                            kufr[kt],
                            start=(kt == 0),
                            stop=(kt == MT - 1),
                        )
                        if kt == 0:
                            nc.vector.tensor_mul(pacc, kufb[kt], pr)
                        else:
                            pk = work.tile([128, CHW], DTB, tag="pk", name="pk")[
                                :, :cw
                            ]
                            nc.vector.tensor_mul(pk, kufb[kt], pr)
                            nc.vector.tensor_add(pacc, pacc, pk)
                    pvp = ps.tile([1, CHW], f32, tag="pvp", name="pvp", bufs=1)[
                        :, :cw
                    ]
                    nc.tensor.matmul(pvp, osb, pacc)
                    nc.vector.tensor_copy(vacc[g][:, sl], pvp)
                    nc.scalar.copy(macc[g][:, sl], pmp)
                nc.sync.dma_start(out=out[g : g + 1, :], in_=macc[g][0:1, :])
                nc.sync.dma_start(out=out[G + g : G + g + 1, :], in_=vacc[g][0:1, :])

    nc.compile()
    return nc


def _get_nc(scale_imm: float, mode: str):
    key = (round(scale_imm, 12), mode)
    if key not in _CACHE:
        _CACHE[key] = _build(scale_imm, mode)
    return _CACHE[key]


def _host_prep(ND_X, Z, q_mu, q_sqrt, variance, lengthscale, mode):
    from numpy.lib.stride_tricks import sliding_window_view

    ls = float(lengthscale)
    var = float(variance)
    scale = -0.5 / (ls * ls)
    ndta, ndtb = _np_dts(mode)

    x = np.asarray(ND_X, np.float32).reshape(N, H, W, C)
    swv = sliding_window_view(x, (PH, PW), axis=(1, 2))  # [N,28,28,C,5,5]
    pats = np.ascontiguousarray(swv.transpose(0, 1, 2, 4, 5, 3)).reshape(N, P, L)
    PNL = np.ascontiguousarray(pats.transpose(1, 0, 2))  # [P,N,L] float32

    Z64 = np.asarray(Z, np.float64)
    zsq = np.einsum("gml,gml->gm", Z64, Z64)  # [G,M]
    sqd = zsq[:, :, None] + zsq[:, None, :] - 2.0 * np.einsum(
        "gml,gnl->gmn", Z64, Z64
    )
    Kuu = var * np.exp(0.5 * sqd / (-ls * ls)) + JITTER * np.eye(M)
    Kinv = np.linalg.inv(Kuu)  # [G,M,M]
    Lq = np.tril(np.asarray(q_sqrt, np.float64))
    Bm = np.einsum("gmn,gnk->gmk", Kinv, Lq)
    Q = Kinv - np.einsum("gmk,gnk->gmn", Bm, Bm)  # [G,M,M]
    d = np.einsum("gmn,ng->gm", Kinv, np.asarray(q_mu, np.float64))  # [G,M]
    bias = scale * zsq + np.log(var)  # [G,M]

    zaug_h = np.ascontiguousarray(
        (-2.0 * Z64).transpose(2, 0, 1)
    ).astype(ndta)  # [L,G,M]
    qmat_h = np.ascontiguousarray(
        Q.reshape(G, MT, 128, M).transpose(2, 0, 1, 3)
    ).astype(ndtb)
    dv_h = np.ascontiguousarray(
        d.reshape(G, MT, 128).transpose(2, 0, 1)
    ).reshape(128, G * MT).astype(ndta)
    bv_h = np.ascontiguousarray(
        bias.reshape(G, MT, 128).transpose(2, 0, 1)
    ).reshape(128, G * MT).astype(np.float32)
    ones_h = np.ones([128, 1], ndtb)

    shared = {
        "zaug": zaug_h,
        "qmat": qmat_h,
        "dv": dv_h,
        "bv": bv_h,
        "ones": ones_h,
    }
    in_maps = []
    cs_all = []  # per-core per-column exp(scale*||x||^2), float64
    for c in range(NCORES):
        Xc = PNL[c * PLOC : (c + 1) * PLOC].reshape(XL, L)
        xt_h = np.ascontiguousarray(Xc.T).astype(ndta)
        xsq = np.einsum(
            "xl,xl->x", Xc.astype(np.float64), Xc.astype(np.float64)
        )
        cs_all.append(np.exp(scale * xsq))
        in_maps.append({"xt": xt_h, **shared})
    return in_maps, cs_all, scale, var


def _run(inputs, trace=False, trace_kwargs=None, mode=None):
    _ensure_concourse()
    from concourse.bass_utils import run_bass_kernel_spmd

    mode = mode or MODE
    in_maps, cs_all, scale, var = _host_prep(**inputs, mode=mode)
    nc = _get_nc(scale, mode)
    bkr = run_bass_kernel_spmd(
        nc,
        in_maps,
        list(range(NCORES)),
        trace=trace,
        **(trace_kwargs or {}),
    )
    mean = np.empty([N, P * G], np.float32)
    varr = np.empty([N, P * G], np.float32)
    for c in range(NCORES):
        o = np.asarray(bkr.results[c]["out"], np.float64)  # [2G, XL]
        cs = cs_all[c]  # [XL]
        m = o[:G] * cs  # [G, XL]
        v = var - o[G:] * (cs * cs)
        mean[:, c * PLOC * G : (c + 1) * PLOC * G] = (
            m.reshape(G, PLOC, N).transpose(2, 1, 0).reshape(N, PLOC * G)
        )
        varr[:, c * PLOC * G : (c + 1) * PLOC * G] = (
            v.reshape(G, PLOC, N).transpose(2, 1, 0).reshape(N, PLOC * G)
        )
    return mean, varr, bkr


def kernel(**inputs):
    mean, varr, _ = _run(inputs, trace=False)
    return mean, varr
